# revision 4
# baseline (speedup 1.0000x reference)
"""Trainium2 Bass kernel for nn_Denoising_ResNet: out = x + conv1x1(box_mean3x3(x)) + b.

Device computes delta = conv1x1(box_sum3x3(x)/9) + b in bf16; the residual
+x is added on the host in f32 (saves a full PE pass and half the HBM
write traffic; x itself is uploaded pre-cast to bf16, halving read traffic).

Sharding: data-parallel over batch (32 samples -> 4 per core x 8 cores).
Per-core layout: 2 stacks of 2 samples -> 128 SBUF partitions each
(= 2 samples x 64 channels).

Math decomposition per chunk of output rows:
  - H-direction 3-tap sum on DVE (bf16, row-shifted adds -> 256B-aligned
    APs -> 2x DVE mode). Image top/bottom rows use the clipped 2-tap sum.
  - W-direction 3-tap sum + 1x1 conv FUSED on PE: 3 accumulating matmuls
    per 4-row PSUM bank against the block-diagonal [128,128] stationary
    weight kron(I2, (W/9)^T), moving operand = hs viewed FLAT with element
    offsets {-1,0,+1}. The +-1 shifts wrap across row boundaries; only
    output columns 0 / W-1 are corrupted and they are overwritten below.
  - ALL matmuls share ONE stationary weight: a single standalone ldweights;
    the per-matmul InstLdweights that tile_legalize inserts are pruned from
    the module before compile (each cost ~146ns serialized with its matmul).
  - Edge columns 0 / W-1: per chunk, 4 tiny matmuls compute
    conv(hs[:,0]+hs[:,1]) / conv(hs[:,W-2]+hs[:,W-1]) into a 1-bank PSUM
    tile; DVE scales by 1.5 (edge-clip count fix), corners by an extra 1.5.
  - Edge rows 0 / H-1: DVE scales the finished PSUM row by 1.5 pre-evac.
  - ScalarE evacuates PSUM -> bf16 SBUF with the conv bias; main tiles
    write columns 1..W-2, the psfix tile writes columns 0 and W-1.

Schedule shaping:
  - Stack 0 starts with an 8-row chunk fed by a small 10-row first load so
    PE starts ~5us in; stack 1 ends with an 8-row chunk + small store so
    the tail chain after the last matmul is short.
  - Loads are split across the two HWDGE rings (sync + scalar) and stores
    go on the SWDGE ring (gpsimd): the SDMA engines round-robin across
    rings at packet granularity, so the first loads get ~half the HBM
    bandwidth instead of 1/8th of it.
"""
from contextlib import ExitStack

import numpy as np

import concourse.bass as bass
import concourse.tile as tile
from concourse import bacc, mybir
from concourse.ap import AP
from concourse.bass_utils import run_bass_kernel_spmd

B, C, H, W = 32, 64, 128, 128
NCORES = 8
PER = B // NCORES  # samples per core
NSTACK = PER // 2  # 2-sample stacks per core
GROUP_ROWS = 4  # rows per matmul accumulation group (512 f32 = 1 bank)
TILE_ROWS = 8  # rows per main PSUM tile (2 banks), 2 groups per tile

# (h0, hc) chunk lists: stack 0 warms the pipeline up with a small first
# chunk; stack 1 cools it down with a small last chunk.
CHUNKS = [
    [(0, 8), (8, 24), (32, 32), (64, 32), (96, 32)],
    [(0, 32), (32, 32), (64, 32), (96, 24), (120, 8)],
]
# x row-boundaries of the quarter loads per stack (halos satisfied:
# chunk (h0,hc) needs x rows [h0-1, h0+hc+1))
LOADS = [[0, 10, 34, 66, 98, 128], [0, 34, 66, 98, 128]]

F32 = mybir.dt.float32
BF16 = mybir.dt.bfloat16
IDENT_FN = mybir.ActivationFunctionType.Identity


def _build_nc() -> bass.Bass:
    nc = bacc.Bacc("TRN2", debug=False)
    x = nc.dram_tensor("x", [PER * C, H, W], BF16, kind="ExternalInput")
    w9t = nc.dram_tensor("w9t", [2 * C, 2 * C], BF16, kind="ExternalInput")
    bias2 = nc.dram_tensor("bias2", [2 * C, 1], F32, kind="ExternalInput")
    y = nc.dram_tensor("y", [PER * C, H, W], BF16, kind="ExternalOutput")
    xap = x.ap()
    yap = y.ap()

    with ExitStack() as ctx:
        tc = ctx.enter_context(tile.TileContext(nc))
        cpool = ctx.enter_context(tc.tile_pool(name="const", bufs=1))
        wt = cpool.tile([128, 128], BF16)
        nc.sync.dma_start(out=wt[:], in_=w9t.ap()[:, :])
        bt = cpool.tile([128, 1], F32)
        nc.sync.dma_start(out=bt[:], in_=bias2.ap()[:, :])

        # the one and only weight load; every matmul reuses it
        nc.tensor.ldweights(wt[:])

        ppool = ctx.enter_context(tc.tile_pool(name="psum", bufs=3, space="PSUM"))
        pfpool = ctx.enter_context(tc.tile_pool(name="psfix", bufs=2, space="PSUM"))
        xpool = ctx.enter_context(tc.tile_pool(name="xin", bufs=2))
        thpool = ctx.enter_context(tc.tile_pool(name="th", bufs=2))
        hspool = ctx.enter_context(tc.tile_pool(name="hs", bufs=2))
        opool = ctx.enter_context(tc.tile_pool(name="out", bufs=3))

        def mm(out_ap, mov_ap, start, stop):
            inst = nc.tensor.matmul(out_ap, wt[:], mov_ap, start=start, stop=stop)
            inst.ldweights = False
            return inst

        # alternate the two HWDGE rings for loads, in need-order
        load_engines = [nc.scalar, nc.sync]
        nload = 0

        nstores = sum(len(c) for c in CHUNKS)
        istore = 0

        for g in range(NSTACK):
            p0 = g * 128
            xt = xpool.tile([128, H, W], BF16)
            lb = LOADS[g]
            for q in range(len(lb) - 1):
                eng = load_engines[nload % 2]
                nload += 1
                eng.dma_start(
                    out=xt[:, lb[q] : lb[q + 1], :],
                    in_=xap[p0 : p0 + 128, lb[q] : lb[q + 1], :],
                )
            for ci, (h0, hc) in enumerate(CHUNKS[g]):
                first = h0 == 0
                last = h0 + hc == H
                ntile = hc // TILE_ROWS

                # H-direction 3-tap sum (DVE 2x mode; row shifts keep APs
                # 4B-aligned). hs data rows 1..hc; rows 0 / hc+1 are pads
                # read only by the wrapping +-1 shifted matmul operands.
                th = thpool.tile([128, hc + 1, W], BF16)
                hs = hspool.tile([128, hc + 2, W], BF16)
                nc.vector.memset(hs[:, 0:1, W - 1 : W], 0.0)
                nc.vector.memset(hs[:, hc + 1 : hc + 2, 0:1], 0.0)
                ja = 1 if first else 0
                jb = hc if last else hc + 1
                nc.vector.tensor_add(
                    th[:, ja:jb, :],
                    xt[:, h0 - 1 + ja : h0 - 1 + jb, :],
                    xt[:, h0 + ja : h0 + jb, :],
                )
                if first:
                    nc.vector.tensor_copy(th[:, 0:1, :], xt[:, 0:1, :])
                ib = hc - 1 if last else hc
                nc.vector.tensor_add(
                    hs[:, 1 : 1 + ib, :],
                    th[:, 0:ib, :],
                    xt[:, h0 + 1 : h0 + 1 + ib, :],
                )
                if last:
                    nc.vector.tensor_copy(
                        hs[:, hc : hc + 1, :], th[:, hc - 1 : hc, :]
                    )

                hall = hs[:]
                hbase = hall.offset
                hstride = hall.ap[0][0]

                ot = opool.tile([128, hc, W], BF16)
                oall = ot[:]

                # edge columns 0 and W-1: conv of the clipped 2-tap W-sum
                # (emitted before the main tiles so it never tails the chunk)
                pf = pfpool.tile([128, hc, 2], F32)
                mm(pf[:, :, 0:1], hs[:, 1 : 1 + hc, 0:1], True, False)
                mm(pf[:, :, 0:1], hs[:, 1 : 1 + hc, 1:2], False, False)
                mm(pf[:, :, 1:2], hs[:, 1 : 1 + hc, W - 2 : W - 1], False, False)
                mm(pf[:, :, 1:2], hs[:, 1 : 1 + hc, W - 1 : W], False, True)
                nc.vector.tensor_scalar_mul(pf[:], pf[:], 1.5)
                if first:
                    nc.vector.tensor_scalar_mul(pf[:, 0:1, :], pf[:, 0:1, :], 1.5)
                if last:
                    nc.vector.tensor_scalar_mul(
                        pf[:, hc - 1 : hc, :], pf[:, hc - 1 : hc, :], 1.5
                    )
                oedge = AP(
                    oall.tensor,
                    oall.offset,
                    [[oall.ap[0][0], 128], [W, hc], [W - 1, 2]],
                )
                nc.scalar.activation(oedge, pf[:], IDENT_FN, bias=bt[:])

                for tp in range(ntile):
                    ps = ppool.tile([128, TILE_ROWS, W], F32, tag="ps")
                    t0 = tp * TILE_ROWS
                    for hp in range(2):
                        a = t0 + hp * GROUP_ROWS
                        ga, gb = hp * GROUP_ROWS, (hp + 1) * GROUP_ROWS
                        for dw in (-1, 0, 1):
                            mov = AP(
                                hall.tensor,
                                hbase + (1 + a) * W + dw,
                                [[hstride, 128], [1, GROUP_ROWS * W]],
                            )
                            mm(ps[:, ga:gb, :], mov, dw == -1, dw == 1)
                    # edge-row count fix (conv part only; bias comes later)
                    if first and tp == 0:
                        nc.vector.tensor_scalar_mul(
                            ps[:, 0:1, :], ps[:, 0:1, :], 1.5
                        )
                    if last and tp == ntile - 1:
                        nc.vector.tensor_scalar_mul(
                            ps[:, TILE_ROWS - 1 : TILE_ROWS, :],
                            ps[:, TILE_ROWS - 1 : TILE_ROWS, :],
                            1.5,
                        )
                    nc.scalar.activation(
                        ot[:, t0 : t0 + TILE_ROWS, 1 : W - 1],
                        ps[:, :, 1 : W - 1],
                        IDENT_FN,
                        bias=bt[:],
                    )

                istore += 1
                seng = nc.sync if istore == nstores else nc.gpsimd
                seng.dma_start(
                    out=yap[p0 : p0 + 128, h0 : h0 + hc, :], in_=ot[:]
                )

    # tile_legalize inserts a bare InstLdweights before every matmul even
    # though every matmul reuses the one stationary weight. Drop all but the
    # first (the explicit one carrying the wt-DMA wait); they have no
    # sync_info so removal is safe.
    for fn in nc.m.functions:
        for blk in fn.blocks:
            insts = list(blk.instructions)
            keep, seen = [], False
            for inst in insts:
                if type(inst).__name__ == "InstLdweights":
                    si = inst.sync_info
                    bare = not (si and (list(si.on_wait) or list(si.on_update)))
                    if seen and bare:
                        continue
                    seen = True
                keep.append(inst)
            if len(keep) != len(insts):
                blk.instructions = keep

    nc.compile()
    return nc


_NC = None


def _get_nc() -> bass.Bass:
    global _NC
    if _NC is None:
        _NC = _build_nc()
    return _NC


def _host_inputs(x: np.ndarray, conv_w: np.ndarray, conv_b: np.ndarray):
    import ml_dtypes

    bf = ml_dtypes.bfloat16
    conv_w = np.asarray(conv_w)
    conv_b = np.asarray(conv_b)
    x = np.ascontiguousarray(np.asarray(x), dtype=np.float32)
    w9t = np.zeros((2 * C, 2 * C), dtype=np.float32)
    wT = (conv_w.astype(np.float32) / 9.0).T
    w9t[0:C, 0:C] = wT
    w9t[C : 2 * C, C : 2 * C] = wT
    w9t = w9t.astype(bf)
    bias2 = np.concatenate([conv_b, conv_b]).reshape(2 * C, 1).astype(np.float32)
    xb = x.astype(bf)
    in_maps = []
    for i in range(NCORES):
        xi = xb[i * PER : (i + 1) * PER].reshape(PER * C, H, W)
        in_maps.append({"x": xi, "w9t": w9t, "bias2": bias2})
    return in_maps


def _combine(res, x: np.ndarray) -> np.ndarray:
    """Gather per-core bf16 delta outputs and add the f32 residual + x."""
    x = np.asarray(x)
    outs = [
        np.asarray(res.results[i]["y"])
        .astype(np.float32)
        .reshape(PER, C, H, W)
        for i in range(NCORES)
    ]
    delta = np.concatenate(outs, axis=0)
    return x.astype(np.float32) + delta


def kernel(x: np.ndarray, conv_w: np.ndarray, conv_b: np.ndarray) -> np.ndarray:
    nc = _get_nc()
    in_maps = _host_inputs(x, conv_w, conv_b)
    res = run_bass_kernel_spmd(nc, in_maps, list(range(NCORES)))
    return _combine(res, x)


# revision 5
# speedup vs baseline: 1.3135x; 1.3135x over previous
"""Trainium2 Bass kernel for nn_Denoising_ResNet: out = x + conv1x1(box_mean3x3(x)) + b.

Device computes delta = conv1x1(box_sum3x3(x)/9) + b in bf16; the residual
+x is added on the host in f32 (saves a full PE pass and half the HBM
write traffic; x itself is uploaded pre-cast to bf16, halving read traffic).

Sharding: data-parallel over batch (32 samples -> 4 per core x 8 cores).
Per-core layout: 2 stacks of 2 samples -> 128 SBUF partitions each
(= 2 samples x 64 channels).

Math decomposition per chunk of output rows:
  - H-direction 3-tap sum on DVE (bf16, row-shifted adds -> 256B-aligned
    APs -> 2x DVE mode). Image top/bottom rows use the clipped 2-tap sum.
  - W-direction 3-tap sum + 1x1 conv FUSED on PE: 3 accumulating matmuls
    per 4-row PSUM bank against the block-diagonal [128,128] stationary
    weight kron(I2, (W/9)^T), moving operand = hs viewed FLAT with element
    offsets {-1,0,+1}. The +-1 shifts wrap across row boundaries; only
    output columns 0 / W-1 are corrupted and they are overwritten below.
  - ALL matmuls share ONE stationary weight: a single standalone ldweights;
    the per-matmul InstLdweights that tile_legalize inserts are pruned from
    the module before compile (each cost ~146ns serialized with its matmul).
  - Edge columns 0 / W-1: per chunk, 4 tiny matmuls compute
    conv(hs[:,0]+hs[:,1]) / conv(hs[:,W-2]+hs[:,W-1]) into a 1-bank PSUM
    tile; DVE scales by 1.5 (edge-clip count fix), corners by an extra 1.5.
  - Edge rows 0 / H-1: DVE scales the finished PSUM row by 1.5 pre-evac.
  - ScalarE evacuates PSUM -> bf16 SBUF with the conv bias; main tiles
    write columns 1..W-2, the psfix tile writes columns 0 and W-1.

Schedule shaping:
  - Stack 0 starts with an 8-row chunk fed by a small 10-row first load so
    PE starts ~5us in; stack 1 ends with an 8-row chunk + small store so
    the tail chain after the last matmul is short.
  - Loads are split across the two HWDGE rings (sync + scalar) and stores
    go on the SWDGE ring (gpsimd): the SDMA engines round-robin across
    rings at packet granularity, so the first loads get ~half the HBM
    bandwidth instead of 1/8th of it.
"""
from contextlib import ExitStack

import numpy as np

import concourse.bass as bass
import concourse.tile as tile
from concourse import bacc, mybir
from concourse.ap import AP
from concourse.bass_utils import run_bass_kernel_spmd

B, C, H, W = 32, 64, 128, 128
NCORES = 8
PER = B // NCORES  # samples per core
NSTACK = PER // 2  # 2-sample stacks per core
GROUP_ROWS = 4  # rows per matmul accumulation group (512 f32 = 1 bank)
TILE_ROWS = 8  # rows per main PSUM tile (2 banks), 2 groups per tile

# (h0, hc) chunk lists: stack 0 warms the pipeline up with a small first
# chunk; stack 1 cools it down with a small last chunk.
CHUNKS = [
    [(0, 8), (8, 24), (32, 32), (64, 32), (96, 32)],
    [(0, 32), (32, 32), (64, 32), (96, 24), (120, 8)],
]
# x row-boundaries of the quarter loads per stack (halos satisfied:
# chunk (h0,hc) needs x rows [h0-1, h0+hc+1))
LOADS = [[0, 10, 34, 66, 98, 128], [0, 34, 66, 98, 128]]

F32 = mybir.dt.float32
BF16 = mybir.dt.bfloat16
IDENT_FN = mybir.ActivationFunctionType.Identity


def _build_nc() -> bass.Bass:
    nc = bacc.Bacc("TRN2", debug=False)
    x = nc.dram_tensor("x", [PER * C, H, W], BF16, kind="ExternalInput")
    w9t = nc.dram_tensor("w9t", [2 * C, 2 * C], BF16, kind="ExternalInput")
    bias2 = nc.dram_tensor("bias2", [2 * C, 1], F32, kind="ExternalInput")
    y = nc.dram_tensor("y", [PER * C, H, W], BF16, kind="ExternalOutput")
    xap = x.ap()
    yap = y.ap()

    with ExitStack() as ctx:
        tc = ctx.enter_context(tile.TileContext(nc))
        cpool = ctx.enter_context(tc.tile_pool(name="const", bufs=1))
        wt = cpool.tile([128, 128], BF16)
        nc.sync.dma_start(out=wt[:], in_=w9t.ap()[:, :])
        bt = cpool.tile([128, 1], F32)
        nc.sync.dma_start(out=bt[:], in_=bias2.ap()[:, :])

        # the one and only weight load; every matmul reuses it
        nc.tensor.ldweights(wt[:])

        ppool = ctx.enter_context(tc.tile_pool(name="psum", bufs=3, space="PSUM"))
        pfpool = ctx.enter_context(tc.tile_pool(name="psfix", bufs=2, space="PSUM"))
        xpool = ctx.enter_context(tc.tile_pool(name="xin", bufs=2))
        thpool = ctx.enter_context(tc.tile_pool(name="th", bufs=2))
        hspool = ctx.enter_context(tc.tile_pool(name="hs", bufs=2))
        opool = ctx.enter_context(tc.tile_pool(name="out", bufs=3))

        def mm(out_ap, mov_ap, start, stop):
            inst = nc.tensor.matmul(out_ap, wt[:], mov_ap, start=start, stop=stop)
            inst.ldweights = False
            return inst

        # all loads on the sync HWDGE ring: within-ring FIFO completion
        # gives exactly the need-order prioritization (measured 384 GB/s)
        nstores = sum(len(c) for c in CHUNKS)
        istore = 0

        for g in range(NSTACK):
            p0 = g * 128
            xt = xpool.tile([128, H, W], BF16)
            lb = LOADS[g]
            for q in range(len(lb) - 1):
                nc.sync.dma_start(
                    out=xt[:, lb[q] : lb[q + 1], :],
                    in_=xap[p0 : p0 + 128, lb[q] : lb[q + 1], :],
                )
            for ci, (h0, hc) in enumerate(CHUNKS[g]):
                first = h0 == 0
                last = h0 + hc == H
                ntile = hc // TILE_ROWS

                # H-direction 3-tap sum (DVE 2x mode; row shifts keep APs
                # 4B-aligned). hs data rows 1..hc; rows 0 / hc+1 are pads
                # read only by the wrapping +-1 shifted matmul operands.
                th = thpool.tile([128, hc + 1, W], BF16)
                hs = hspool.tile([128, hc + 2, W], BF16)
                nc.vector.memset(hs[:, 0:1, W - 1 : W], 0.0)
                nc.vector.memset(hs[:, hc + 1 : hc + 2, 0:1], 0.0)
                ja = 1 if first else 0
                jb = hc if last else hc + 1
                nc.vector.tensor_add(
                    th[:, ja:jb, :],
                    xt[:, h0 - 1 + ja : h0 - 1 + jb, :],
                    xt[:, h0 + ja : h0 + jb, :],
                )
                if first:
                    nc.vector.tensor_copy(th[:, 0:1, :], xt[:, 0:1, :])
                ib = hc - 1 if last else hc
                nc.vector.tensor_add(
                    hs[:, 1 : 1 + ib, :],
                    th[:, 0:ib, :],
                    xt[:, h0 + 1 : h0 + 1 + ib, :],
                )
                if last:
                    nc.vector.tensor_copy(
                        hs[:, hc : hc + 1, :], th[:, hc - 1 : hc, :]
                    )

                hall = hs[:]
                hbase = hall.offset
                hstride = hall.ap[0][0]

                ot = opool.tile([128, hc, W], BF16)
                oall = ot[:]

                # edge columns 0 and W-1: conv of the clipped 2-tap W-sum
                # (emitted before the main tiles so it never tails the chunk)
                pf = pfpool.tile([128, hc, 2], F32)
                mm(pf[:, :, 0:1], hs[:, 1 : 1 + hc, 0:1], True, False)
                mm(pf[:, :, 0:1], hs[:, 1 : 1 + hc, 1:2], False, False)
                mm(pf[:, :, 1:2], hs[:, 1 : 1 + hc, W - 2 : W - 1], False, False)
                mm(pf[:, :, 1:2], hs[:, 1 : 1 + hc, W - 1 : W], False, True)
                nc.vector.tensor_scalar_mul(pf[:], pf[:], 1.5)
                if first:
                    nc.vector.tensor_scalar_mul(pf[:, 0:1, :], pf[:, 0:1, :], 1.5)
                if last:
                    nc.vector.tensor_scalar_mul(
                        pf[:, hc - 1 : hc, :], pf[:, hc - 1 : hc, :], 1.5
                    )
                oedge = AP(
                    oall.tensor,
                    oall.offset,
                    [[oall.ap[0][0], 128], [W, hc], [W - 1, 2]],
                )
                nc.scalar.activation(oedge, pf[:], IDENT_FN, bias=bt[:])

                for tp in range(ntile):
                    ps = ppool.tile([128, TILE_ROWS, W], F32, tag="ps")
                    t0 = tp * TILE_ROWS
                    for hp in range(2):
                        a = t0 + hp * GROUP_ROWS
                        ga, gb = hp * GROUP_ROWS, (hp + 1) * GROUP_ROWS
                        for dw in (-1, 0, 1):
                            mov = AP(
                                hall.tensor,
                                hbase + (1 + a) * W + dw,
                                [[hstride, 128], [1, GROUP_ROWS * W]],
                            )
                            mm(ps[:, ga:gb, :], mov, dw == -1, dw == 1)
                    # edge-row count fix (conv part only; bias comes later)
                    if first and tp == 0:
                        nc.vector.tensor_scalar_mul(
                            ps[:, 0:1, :], ps[:, 0:1, :], 1.5
                        )
                    if last and tp == ntile - 1:
                        nc.vector.tensor_scalar_mul(
                            ps[:, TILE_ROWS - 1 : TILE_ROWS, :],
                            ps[:, TILE_ROWS - 1 : TILE_ROWS, :],
                            1.5,
                        )
                    nc.scalar.activation(
                        ot[:, t0 : t0 + TILE_ROWS, 1 : W - 1],
                        ps[:, :, 1 : W - 1],
                        IDENT_FN,
                        bias=bt[:],
                    )

                istore += 1
                seng = nc.sync if istore == nstores else nc.gpsimd
                seng.dma_start(
                    out=yap[p0 : p0 + 128, h0 : h0 + hc, :], in_=ot[:]
                )

    # tile_legalize inserts a bare InstLdweights before every matmul even
    # though every matmul reuses the one stationary weight. Drop all but the
    # first (the explicit one carrying the wt-DMA wait); they have no
    # sync_info so removal is safe.
    for fn in nc.m.functions:
        for blk in fn.blocks:
            insts = list(blk.instructions)
            keep, seen = [], False
            for inst in insts:
                if type(inst).__name__ == "InstLdweights":
                    si = inst.sync_info
                    bare = not (si and (list(si.on_wait) or list(si.on_update)))
                    if seen and bare:
                        continue
                    seen = True
                keep.append(inst)
            if len(keep) != len(insts):
                blk.instructions = keep

    nc.compile()
    return nc


_NC = None


def _get_nc() -> bass.Bass:
    global _NC
    if _NC is None:
        _NC = _build_nc()
    return _NC


def _host_inputs(x: np.ndarray, conv_w: np.ndarray, conv_b: np.ndarray):
    import ml_dtypes

    bf = ml_dtypes.bfloat16
    conv_w = np.asarray(conv_w)
    conv_b = np.asarray(conv_b)
    x = np.ascontiguousarray(np.asarray(x), dtype=np.float32)
    w9t = np.zeros((2 * C, 2 * C), dtype=np.float32)
    wT = (conv_w.astype(np.float32) / 9.0).T
    w9t[0:C, 0:C] = wT
    w9t[C : 2 * C, C : 2 * C] = wT
    w9t = w9t.astype(bf)
    bias2 = np.concatenate([conv_b, conv_b]).reshape(2 * C, 1).astype(np.float32)
    xb = x.astype(bf)
    in_maps = []
    for i in range(NCORES):
        xi = xb[i * PER : (i + 1) * PER].reshape(PER * C, H, W)
        in_maps.append({"x": xi, "w9t": w9t, "bias2": bias2})
    return in_maps


def _combine(res, x: np.ndarray) -> np.ndarray:
    """Gather per-core bf16 delta outputs and add the f32 residual + x."""
    x = np.asarray(x)
    outs = [
        np.asarray(res.results[i]["y"])
        .astype(np.float32)
        .reshape(PER, C, H, W)
        for i in range(NCORES)
    ]
    delta = np.concatenate(outs, axis=0)
    return x.astype(np.float32) + delta


def kernel(x: np.ndarray, conv_w: np.ndarray, conv_b: np.ndarray) -> np.ndarray:
    nc = _get_nc()
    in_maps = _host_inputs(x, conv_w, conv_b)
    res = run_bass_kernel_spmd(nc, in_maps, list(range(NCORES)))
    return _combine(res, x)
